# revision 2
# baseline (speedup 1.0000x reference)
"""Trainium2 Bass kernel for nn_BoundaryLoss (boundary-weighted NLL loss).

Contract: kernel(**inputs) takes FULL inputs (logits (8,20,512,512) f32,
targets (8,512,512) int), returns the FULL output (scalar f32 mean loss).
Internally shards batch across 8 NeuronCores (1 image per core), runs an
SPMD Bass program, and reduces the per-core partial sums on the host.

Math per image:
  boundaries = max_c sobel_mag(onehot(targets))   (3x3, replicate pad)
  weight = exp(clip(3*boundaries, 0, 5))
  nll    = logsumexp_c(logits) - logits[targets]
  out    = mean(weight * nll)

Structure:
  * softmax sums (phase A) on class-on-partition layout, S1/S2 via PE,
    staged to DRAM in bf16 (s_scr).
  * boundary weights (phase B) from 28 pairwise equality maps of
    row/col-shifted target stacks built purely in SBUF (partition-shifted
    SBUF->SBUF DMAs, no DRAM bounce); min(mag^2,3) staged to DRAM (m3_scr).
  * final combine (phase C) streams s_scr+m3_scr back in field-major
    [126,512] pieces that unlock progressively while phase A still runs.
  * phases A/B/C are INTERLEAVED in emission order so every engine's
    in-order queue mixes all three workloads.
  * HW-specific: no gpsimd SWDGE DMAs (slow descriptor generation), and
    the Activation engine only ever runs Exp/Ln/Copy (function-table
    switches force costly table reloads on silicon).
"""

import math
import os
import sys

import numpy as np

sys.path.insert(0, "/opt/trn_rl_repo")

import concourse.bass as bass  # noqa: E402
import concourse.tile as tile  # noqa: E402
from concourse import bacc, mybir  # noqa: E402
from concourse.bass_utils import run_bass_kernel_spmd  # noqa: E402

FP32 = mybir.dt.float32
BF16 = mybir.dt.bfloat16
Alu = mybir.AluOpType
Act = mybir.ActivationFunctionType

H = W = 512
C = 20
HW = H * W
B = 8
NCORES = 8
NRB = H // 128          # row blocks
NG = 6                  # class-layout groups
FG = 43691              # pixels per group (group 5 is 2 pixels short)
HWP = HW + 2            # padded scratch row (absorbs group-5 tail garbage)
PADIN = 8               # host-side padding of flat inputs
SM_CHUNK = 512 * int(os.environ.get("KSMC3", "3"))  # softmax pixel chunk
MMF = 512               # matmul free width
CW = int(os.environ.get("KCW", "256"))  # boundary column chunk
KBEND = int(os.environ.get("KBEND", "16"))  # last A-chunk with B pieces

# phase C field-major pieces: (f0, parts, inner, X) -> tile [parts, ...]
# covering pixels {g*FG + f0 + q*X + x} for g<6, q<inner, x<X
PIECES = [(j * 10752, 126, 21, 512) for j in range(4)] + [(43008, 6, 1, 683)]

NEI = {
    "NW": (-1, -1, -1, -1), "N": (-1, 0, 0, -2), "NE": (-1, 1, 1, -1),
    "W": (0, -1, -2, 0), "E": (0, 1, 2, 0),
    "SW": (1, -1, -1, 1), "S": (1, 0, 0, 2), "SE": (1, 1, 1, 1),
}
KS = list(NEI)

# weight cubic through m3 in {0,1,2,3} -> {1, e^3, e^(3*sqrt2), e^5}
_WV = [1.0, math.exp(3.0), math.exp(3.0 * math.sqrt(2.0)), math.exp(5.0)]
_VAND = np.array([[m ** p for p in range(4)] for m in range(4)], np.float64)
PC = np.linalg.solve(_VAND, np.array(_WV, np.float64))  # c0..c3

PAT_ASM = os.environ.get("KPASM", "DDDDP")      # per-k adds/subs/mults
PAT_SQ = os.environ.get("KPSQ", "D")         # squares x2,y2
PAT_OH = os.environ.get("KPOH", "D")          # one-hot slices
PAT_EO = os.environ.get("KPEO", "DP")           # eo multiply per chunk
PAT_EV = os.environ.get("KPEV", "AD")           # psum evacuation per chunk


def _pair_key(a, b):
    return tuple(sorted((a, b)))


def host_consts():
    import ml_dtypes
    c_col = (np.arange(120, dtype=np.float32) % 20).reshape(120, 1)
    c_coln = -c_col
    lhs_rep = np.zeros((NG, 120), ml_dtypes.bfloat16)
    lhs_s1 = np.zeros((120, 32), ml_dtypes.bfloat16)
    lhs_s2 = np.zeros((120, 32), ml_dtypes.bfloat16)
    for g in range(NG):
        lhs_rep[g, 20 * g:20 * (g + 1)] = 1
        lhs_s1[20 * g:20 * (g + 1), g] = 1
        lhs_s2[20 * g:20 * (g + 1), 6 + g] = 1
    return {"c_col": c_col, "c_coln": c_coln, "lhs_rep": lhs_rep,
            "lhs_s1": lhs_s1, "lhs_s2": lhs_s2}


def make_in_maps(logits, targets):
    """Host-side input prep shared by kernel() and the timing harness."""
    import ml_dtypes
    cmaps = host_consts()
    pad = np.zeros(PADIN, np.float32)
    tpadv = np.zeros(PADIN, ml_dtypes.bfloat16)
    return [
        {"logits": np.concatenate([logits[b].reshape(-1), pad]),
         "tbf": np.concatenate(
             [targets[b].reshape(-1).astype(ml_dtypes.bfloat16), tpadv]),
         **cmaps}
        for b in range(NCORES)
    ]


def build_nc():
    nc = bacc.Bacc("TRN2", target_bir_lowering=False, debug=False)
    logits = nc.dram_tensor("logits", [C * HW + PADIN], FP32,
                            kind="ExternalInput")
    tbf = nc.dram_tensor("tbf", [HW + PADIN], BF16, kind="ExternalInput")
    consts = {
        "c_col": nc.dram_tensor("c_col", [120, 1], FP32,
                                kind="ExternalInput"),
        "c_coln": nc.dram_tensor("c_coln", [120, 1], FP32,
                                 kind="ExternalInput"),
        "lhs_rep": nc.dram_tensor("lhs_rep", [NG, 120], BF16,
                                  kind="ExternalInput"),
        "lhs_s1": nc.dram_tensor("lhs_s1", [120, 32], BF16,
                                 kind="ExternalInput"),
        "lhs_s2": nc.dram_tensor("lhs_s2", [120, 32], BF16,
                                 kind="ExternalInput"),
    }
    out_partial = nc.dram_tensor("out_partial", [128, 8], FP32,
                                 kind="ExternalOutput")
    s_scr = nc.dram_tensor("s_scr", [2, HWP], BF16)
    m3_scr = nc.dram_tensor("m3_scr", [HWP], BF16)

    nrep = int(os.environ.get("KREPEAT", "1"))
    with tile.TileContext(nc) as tc:
        for _ in range(nrep):
            _body(tc, nc, logits, tbf, out_partial, m3_scr, s_scr, consts)
    nc.compile()
    return nc


class Rot:
    """Weighted engine rotation from a pattern string."""

    def __init__(self, nc, pat):
        self.nc = nc
        self.pat = pat
        self.i = 0

    def next(self):
        ch = self.pat[self.i % len(self.pat)]
        self.i += 1
        if ch == "P":
            return self.nc.gpsimd, "P"
        if ch == "A":
            return self.nc.scalar, "A"
        return self.nc.vector, "D"


def _body(tc, nc, logits, tbf, out_partial, m3_scr, s_scr, consts):
    import contextlib
    ctx = contextlib.ExitStack()
    pool = ctx.enter_context(tc.tile_pool(name="main", bufs=1))
    tmp = ctx.enter_context(tc.tile_pool(name="tmp", bufs=8))
    psum = ctx.enter_context(
        tc.tile_pool(name="psum", bufs=1, space="PSUM"))

    PH = os.environ.get("KPHASES", "0ABCR")

    # ---------------- phase 0: consts + target stacks in SBUF --------
    c_col = pool.tile([120, 1], FP32, tag="cCol")
    nc.sync.dma_start(c_col[:], consts["c_col"].ap())
    c_coln = pool.tile([120, 1], FP32, tag="cColN")
    nc.sync.dma_start(c_coln[:], consts["c_coln"].ap())
    lhs_rep = pool.tile([NG, 120], BF16, tag="lhsRep")
    nc.sync.dma_start(lhs_rep[:], consts["lhs_rep"].ap())
    lhs_s1 = pool.tile([120, 32], BF16, tag="lhsS1")
    nc.sync.dma_start(lhs_s1[:], consts["lhs_s1"].ap())
    lhs_s2 = pool.tile([120, 32], BF16, tag="lhsS2")
    nc.sync.dma_start(lhs_s2[:], consts["lhs_s2"].ap())

    # target stacks: rows {-1, 0, +1}, cols -4..515, in BOTH parities
    # (real HW runs DVE bf16 ops at half rate from odd element offsets,
    # so keep even-offset duplicates); built from one HBM read +
    # partition-shifted SBUF->SBUF copies.
    stacks = {}
    if "B" in PH:
        sa0 = pool.tile([128, NRB, 520], BF16, tag="stA0")
        nc.sync.dma_start(
            sa0[:, :, 4:516],
            bass.AP(tensor=tbf, offset=0,
                    ap=[[512, 128], [65536, NRB], [1, 512]]))
        for j in range(4):
            nc.sync.dma_start(
                sa0[:, :, j:j + 1],
                bass.AP(tensor=tbf, offset=0,
                        ap=[[512, 128], [65536, NRB], [1, 1]]))
            nc.sync.dma_start(
                sa0[:, :, 516 + j:517 + j],
                bass.AP(tensor=tbf, offset=511,
                        ap=[[512, 128], [65536, NRB], [1, 1]]))
        sb0 = pool.tile([128, NRB, 519], BF16, tag="stB0")
        nc.sync.dma_start(sb0[:], sa0[:, :, 1:520])
        saM = pool.tile([128, NRB, 520], BF16, tag="stAM")
        nc.sync.dma_start(saM[1:128, :, :], sa0[0:127, :, :])
        nc.sync.dma_start(saM[0:1, 1:NRB, :], sa0[127:128, 0:NRB - 1, :])
        nc.sync.dma_start(saM[0:1, 0:1, :], sa0[0:1, 0:1, :])
        sbM = pool.tile([128, NRB, 519], BF16, tag="stBM")
        nc.sync.dma_start(sbM[1:128, :, :], sb0[0:127, :, :])
        nc.sync.dma_start(sbM[0:1, 1:NRB, :], sb0[127:128, 0:NRB - 1, :])
        nc.sync.dma_start(sbM[0:1, 0:1, :], sb0[0:1, 0:1, :])
        saP = pool.tile([128, NRB, 520], BF16, tag="stAP")
        nc.sync.dma_start(saP[0:127, :, :], sa0[1:128, :, :])
        nc.sync.dma_start(saP[127:128, 0:NRB - 1, :], sa0[0:1, 1:NRB, :])
        nc.sync.dma_start(saP[127:128, NRB - 1:NRB, :],
                          sa0[127:128, NRB - 1:NRB, :])
        sbP = pool.tile([128, NRB, 519], BF16, tag="stBP")
        nc.sync.dma_start(sbP[0:127, :, :], sb0[1:128, :, :])
        nc.sync.dma_start(sbP[127:128, 0:NRB - 1, :], sb0[0:1, 1:NRB, :])
        nc.sync.dma_start(sbP[127:128, NRB - 1:NRB, :],
                          sb0[127:128, NRB - 1:NRB, :])
        stacks = {-1: (saM, sbM), 0: (sa0, sb0), 1: (saP, sbP)}

    def stack_ap(k, c0, cw):
        dy, dx, _, _ = NEI[k]
        sa, sb = stacks[dy]
        if (dx + c0) % 2 == 0:
            off = 4 + dx + c0
            return sa[:, :, off:off + cw]
        off = 3 + dx + c0
        return sb[:, :, off:off + cw]

    rot_oh = Rot(nc, PAT_OH)
    rot_eo = Rot(nc, PAT_EO)
    rot_ev = Rot(nc, PAT_EV)
    rot_asm = Rot(nc, PAT_ASM)
    rot_sq = Rot(nc, PAT_SQ)
    GP_OK = {Alu.add, Alu.subtract, Alu.mult}

    def eng_tt(rot, op):
        if op not in GP_OK:
            return nc.vector
        eng, which = rot.next()
        if which == "A":
            return nc.vector  # ACT cannot do tensor-tensor
        return eng

    # ---------------- phase A: softmax sums (class-on-partition) -----
    def emit_a(cf0):
        cf = min(SM_CHUNK, FG - cf0)
        xbufs = int(os.environ.get("KXBUFS", "3"))
        abufs = int(os.environ.get("KABUFS", "2"))
        x_ck = pool.tile([120, cf], FP32, tag="xck", bufs=xbufs)
        nc.sync.dma_start(x_ck[:],
                          bass.AP(tensor=logits, offset=cf0,
                                  ap=[[FG, NG], [HW, C], [1, cf]]))

        e_ck = pool.tile([120, cf], BF16, tag="eck", bufs=abufs)
        nc.scalar.activation(e_ck[:], x_ck[:], Act.Exp)

        t6 = pool.tile([NG, cf], BF16, tag="t6", bufs=3)
        nc.scalar.dma_start(t6[:], bass.AP(tensor=tbf, offset=cf0,
                                           ap=[[FG, NG], [1, cf]]))

        oh_ck = pool.tile([120, cf], BF16, tag="ohck", bufs=abufs)
        for m0 in range(0, cf, MMF):
            mf = min(MMF, cf - m0)
            trep = psum.tile([120, mf], FP32, tag="trep", bufs=3)
            nc.tensor.matmul(trep[:], lhs_rep[:], t6[:, m0:m0 + mf])
            _, which = rot_oh.next()
            if which == "A":
                # |t-c| then relu(1-|t-c|) on the Activation engine
                a1 = pool.tile([120, mf], BF16, tag="oha1", bufs=2)
                nc.scalar.activation(a1[:], trep[:], Act.Abs,
                                     bias=c_coln[:])
                nc.scalar.activation(oh_ck[:, m0:m0 + mf], a1[:],
                                     Act.Relu, bias=1.0, scale=-1.0)
            else:
                # one-hot: (t_rep == class(partition)); PSUM src -> DVE
                nc.vector.tensor_scalar(oh_ck[:, m0:m0 + mf], trep[:],
                                        c_col[:], None, Alu.is_equal)

        eo_ck = pool.tile([120, cf], BF16, tag="eock", bufs=abufs)
        eng_mul, _ = rot_eo.next()
        eng_mul.tensor_mul(eo_ck[:], e_ck[:], oh_ck[:])

        # S1/S2 sums -> psum rows 32s..32s+11 (6 S1 then 6 S2 per slice)
        ns = (cf + MMF - 1) // MMF
        s12 = psum.tile([96, MMF], FP32, tag="s12", bufs=3)
        for s in range(ns):
            m0 = s * MMF
            mf = min(MMF, cf - m0)
            nc.tensor.matmul(s12[32 * s:32 * s + 32, 0:mf], lhs_s1[:],
                             e_ck[:, m0:m0 + mf], start=True, stop=False)
            nc.tensor.matmul(s12[32 * s:32 * s + 32, 0:mf], lhs_s2[:],
                             eo_ck[:, m0:m0 + mf], start=False, stop=True)
        # evacuation is emitted with a one-chunk lag (software pipelining
        # of the emission order) so the evac op never stalls its engine's
        # stream waiting on this chunk's matmuls.
        def emit_evac():
            s_sb = pool.tile([96, MMF], BF16, tag="ssb", bufs=2)
            eng_ev, _ = rot_ev.next()

            def _cp(dst, src_, eng=eng_ev):
                if eng is nc.scalar:
                    eng.copy(dst, src_)
                else:
                    eng.tensor_copy(dst, src_)
            if cf == ns * MMF:
                _cp(s_sb[0:32 * ns, :], s12[0:32 * ns, :])
            else:
                _cp(s_sb[0:32 * (ns - 1), :], s12[0:32 * (ns - 1), :])
                mf_l = cf - (ns - 1) * MMF
                _cp(s_sb[32 * (ns - 1):32 * ns, 0:mf_l],
                    s12[32 * (ns - 1):32 * ns, 0:mf_l])
            # SBUF -> DRAM scratch (bf16), flat pixel-major. Keep these
            # writes OFF the SP queue: SP carries only the always-ready
            # x_ck prefetch reads (a waiting DMA head blocks its queue).
            for s in range(ns):
                m0 = s * MMF
                mf = min(MMF, cf - m0)
                dst = bass.AP(tensor=s_scr, offset=cf0 + m0,
                              ap=[[HWP, 2], [FG, NG], [1, mf]])
                eng_d = nc.scalar if s % 2 == 0 else nc.sync
                eng_d.dma_start(dst, s_sb[32 * s:32 * s + 12, 0:mf])
        return emit_evac

    a_chunks = list(range(0, FG, SM_CHUNK)) if "A" in PH else []

    # ---------------- phase B: boundary weights ----------------
    def tnew(tag, bufs=2):
        return tmp.tile([128, NRB, CW], BF16, tag=tag, bufs=bufs,
                        name=tag)

    bstate = {}

    def emit_maps(c0, half):
        st = bstate.setdefault(c0, {"maps": {}, "mrun": None})
        todo = []
        for i, a in enumerate(KS):
            for b_ in KS[i + 1:]:
                todo.append(_pair_key(a, b_))
        lo, hi = (0, 14) if half == 0 else (14, 28)
        for a, b_ in todo[lo:hi]:
            mp = tmp.tile([128, NRB, CW], BF16, tag=f"map{a}{b_}", bufs=1)
            # is_equal only exists on DVE (Pool: add/sub/mult only)
            nc.vector.tensor_tensor(
                mp[:], stack_ap(a, c0, CW), stack_ap(b_, c0, CW),
                Alu.is_equal)
            st["maps"][(a, b_)] = mp

    def emit_k(c0, k):
        st = bstate[c0]
        maps = st["maps"]

        def emap(a, b_):
            if a == b_:
                return None
            return maps[_pair_key(a, b_)]

        def signed_diff(k, lp, lm, tag):
            """(tile, const) ~ e[k,lp] - e[k,lm], self maps -> const."""
            tp, tm = emap(k, lp), emap(k, lm)
            if tp is None:                      # 1 - e[k,lm]
                t = tnew(tag, bufs=1)
                nc.vector.tensor_scalar(t[:], tm[:], -1.0, 1.0,
                                        Alu.mult, Alu.add)
                return t, 0.0
            if tm is None:                      # e[k,lp] - 1
                return tp, -1.0
            t = tnew(tag, bufs=1)
            eng_tt(rot_asm, Alu.subtract).tensor_tensor(
                t[:], tp[:], tm[:], Alu.subtract)
            return t, 0.0

        # corner diffs: P = e[k,SE]-e[k,NW], Q = e[k,NE]-e[k,SW]
        tP, cP = signed_diff(k, "SE", "NW", "cdP")
        tQ, cQ = signed_diff(k, "NE", "SW", "cdQ")
        # X = (P+Q) + 2*(e[k,E]-e[k,W]);  Y = (P-Q) + 2*(e[k,S]-e[k,N])
        tE, cE = signed_diff(k, "E", "W", "cdE")
        tS, cS = signed_diff(k, "S", "N", "cdS")
        pq_s = tnew("pqs")
        eng_tt(rot_asm, Alu.add).tensor_tensor(pq_s[:], tP[:], tQ[:],
                                               Alu.add)
        pq_d = tnew("pqd")
        eng_tt(rot_asm, Alu.subtract).tensor_tensor(pq_d[:], tP[:],
                                                    tQ[:], Alu.subtract)
        ex2 = tnew("ex2")
        nc.vector.tensor_scalar(ex2[:], tE[:], 2.0,
                                cP + cQ + 2.0 * cE, Alu.mult, Alu.add)
        ey2 = tnew("ey2")
        nc.vector.tensor_scalar(ey2[:], tS[:], 2.0,
                                cP - cQ + 2.0 * cS, Alu.mult, Alu.add)
        xk = tnew("xk")
        eng_tt(rot_asm, Alu.add).tensor_tensor(xk[:], pq_s[:], ex2[:],
                                               Alu.add)
        yk = tnew("yk")
        eng_tt(rot_asm, Alu.add).tensor_tensor(yk[:], pq_d[:], ey2[:],
                                               Alu.add)
        x2 = tnew("x2")
        eng, which = rot_sq.next()
        if which == "A":
            nc.scalar.activation(x2[:], xk[:], Act.Square)
        else:
            eng.tensor_tensor(x2[:], xk[:], xk[:], Alu.mult)
        y2 = tnew("y2")
        eng, which = rot_sq.next()
        if which == "A":
            nc.scalar.activation(y2[:], yk[:], Act.Square)
        else:
            eng.tensor_tensor(y2[:], yk[:], yk[:], Alu.mult)
        mk = tnew("mk")
        eng_tt(rot_asm, Alu.add).tensor_tensor(mk[:], x2[:], y2[:],
                                               Alu.add)
        if st["mrun"] is None:
            st["mrun"] = mk
        else:
            m2 = tnew("mrun")
            eng_tt(rot_asm, Alu.max).tensor_tensor(m2[:], st["mrun"][:],
                                                   mk[:], Alu.max)
            st["mrun"] = m2

    def emit_min(c0):
        m3c = tnew(f"m3c{c0}", bufs=1)
        nc.vector.tensor_single_scalar(m3c[:], bstate[c0]["mrun"][:],
                                       3.0, Alu.min)
        nc.scalar.dma_start(
            bass.AP(tensor=m3_scr, offset=c0,
                    ap=[[512, 128], [65536, NRB], [1, CW]]),
            m3c[:])
        del bstate[c0]

    b_pieces = []
    for c0 in (range(0, W, CW) if "B" in PH else []):
        b_pieces.append(lambda c0=c0: emit_maps(c0, 0))
        b_pieces.append(lambda c0=c0: emit_maps(c0, 1))
        for k in KS:
            b_pieces.append(lambda c0=c0, k=k: emit_k(c0, k))
        b_pieces.append(lambda c0=c0: emit_min(c0))

    # ---------------- phase C: combine (field-major pieces) ----------
    if "B" not in PH:
        m3i = pool.tile([128, 2048], BF16, tag="m3i", name="m3i")
        nc.vector.memset(m3i[:], 3.0)
        nc.sync.dma_start(
            bass.AP(tensor=m3_scr, offset=0, ap=[[2048, 128], [1, 2048]]),
            m3i[:])
        nc.sync.dma_start(
            bass.AP(tensor=m3_scr, offset=HW, ap=[[1, 2]]),
            m3i[0:1, 0:2])
    else:
        m3i = pool.tile([1, 2], BF16, tag="m3i", name="m3i")
        nc.vector.memset(m3i[:], 0.0)
        nc.sync.dma_start(
            bass.AP(tensor=m3_scr, offset=HW, ap=[[1, 2]]), m3i[:])
    if "A" not in PH and "C" in PH:
        s_scr_init = pool.tile([128, 4096], BF16, tag="sinit",
                               name="sinit")
        nc.vector.memset(s_scr_init[:], 1.0)
        nc.sync.dma_start(
            bass.AP(tensor=s_scr, offset=0, ap=[[4096, 128], [1, 4096]]),
            s_scr_init[:])
        nc.sync.dma_start(
            bass.AP(tensor=s_scr, offset=4096 * 128, ap=[[1, 4]]),
            s_scr_init[0:1, 0:4])

    accs = []

    def emit_c(pi):
        f0, parts, inner, X = PIECES[pi]
        fl = inner * X
        ap_s1 = bass.AP(tensor=s_scr, offset=f0,
                        ap=[[FG, NG], [X, inner], [1, X]])
        ap_s2 = bass.AP(tensor=s_scr, offset=HWP + f0,
                        ap=[[FG, NG], [X, inner], [1, X]])
        ap_m3 = bass.AP(tensor=m3_scr, offset=f0,
                        ap=[[FG, NG], [X, inner], [1, X]])
        shp = [parts, X] if inner == 1 else [parts, X]
        s1t = pool.tile(shp, BF16, tag=f"cs1_{parts}", bufs=2, name="cs1")
        nc.scalar.dma_start(s1t[:], ap_s1)
        s2t = pool.tile(shp, BF16, tag=f"cs2_{parts}", bufs=2, name="cs2")
        nc.scalar.dma_start(s2t[:], ap_s2)
        m3t = pool.tile(shp, BF16, tag=f"cm3_{parts}", bufs=2, name="cm3")
        nc.sync.dma_start(m3t[:], ap_m3)
        nc.scalar.activation(s1t[:], s1t[:], Act.Ln)
        nc.scalar.activation(s2t[:], s2t[:], Act.Ln)
        u = s1t
        nc.vector.tensor_sub(u[:], s1t[:], s2t[:])
        h1 = pool.tile(shp, BF16, tag=f"ch1_{parts}", bufs=2, name="ch1")
        nc.vector.tensor_scalar(h1[:], m3t[:], float(PC[3]), float(PC[2]),
                                Alu.mult, Alu.add)
        h2 = pool.tile(shp, BF16, tag=f"ch2_{parts}", bufs=2, name="ch2")
        nc.vector.tensor_tensor(h2[:], h1[:], m3t[:], Alu.mult)
        nc.vector.tensor_scalar(h1[:], h2[:], 1.0, float(PC[1]),
                                Alu.mult, Alu.add)
        nc.vector.tensor_tensor(h2[:], h1[:], m3t[:], Alu.mult)
        nc.vector.tensor_scalar(h1[:], h2[:], 1.0, float(PC[0]),
                                Alu.mult, Alu.add)
        accp = pool.tile([parts, 1], FP32, tag=f"acc{pi}", name="accp")
        if "R" in PH:
            nc.vector.scalar_tensor_tensor(h2[:], h1[:], 0.0, u[:],
                                           Alu.bypass, Alu.mult,
                                           accum_out=accp[:])
        else:
            nc.vector.memset(accp[:], 0.0)
        nc.scalar.dma_start(
            bass.AP(tensor=out_partial, offset=pi,
                    ap=[[8, parts], [1, 1]]),
            accp[:])
        accs.append(accp)

    # C piece pi is gated by A chunk (7*(pi+1)-1) and by all of B.
    c_gate = {}
    if "C" in PH:
        n_ac = len(a_chunks)
        for pi in range(len(PIECES)):
            gate = 7 * (pi + 1) - 1 if pi < 4 else n_ac - 1
            gate = max(gate, min(KBEND, n_ac - 1))
            c_gate.setdefault(min(gate, max(n_ac - 1, 0)), []).append(pi)

    # ------------- interleaved emission of phases A, B, C -------------
    n_a, n_b = len(a_chunks), len(b_pieces)
    done_b = 0
    nbe = min(KBEND, max(n_a, 1))
    for i, cf0 in enumerate(a_chunks):
        ev = emit_a(cf0)
        ev()
        target = (i + 1) * n_b // nbe
        while done_b < min(target, n_b):
            b_pieces[done_b]()
            done_b += 1
        for pi in c_gate.pop(i, []):
            emit_c(pi)
    while done_b < n_b:
        b_pieces[done_b]()
        done_b += 1
    for i in sorted(c_gate):
        for pi in c_gate[i]:
            emit_c(pi)

    if "C" not in PH:
        acc0 = pool.tile([128, 8], FP32, tag="acc0", name="acc0")
        nc.vector.memset(acc0[:], 0.0)
        nc.sync.dma_start(out_partial.ap(), acc0[:])
    ctx.close()


_NC_CACHE = None


def _get_nc():
    global _NC_CACHE
    if _NC_CACHE is None:
        _NC_CACHE = build_nc()
    return _NC_CACHE


def kernel(logits, targets):
    logits = np.ascontiguousarray(np.asarray(logits, dtype=np.float32))
    targets = np.ascontiguousarray(np.asarray(targets)).astype(np.int32)
    assert logits.shape == (B, C, H, W), logits.shape
    assert targets.shape == (B, H, W), targets.shape

    nc = _get_nc()
    in_maps = make_in_maps(logits, targets)
    res = run_bass_kernel_spmd(nc, in_maps, list(range(NCORES)))
    total = 0.0
    for r in res.results:
        total += float(np.asarray(r["out_partial"], np.float64).sum())
    return np.float32(total / (B * H * W))
